# revision 3
# baseline (speedup 1.0000x reference)
"""Trainium2 Bass kernel for the AttentionBlock problem.

Reference semantics (shapes hardcoded):
    x [4, 256, 64, 64]; 1x1-conv weights q_w/k_w/v_w [256, 258] (+biases),
    fc_w [256, 256], fc_b [256].
    x0 = concat(x, pos) -> [B, 258, 4096]
    q/k/v = relu(W @ x0 + b)                    [B, 256, 4096]
    attn  = softmax_causal(q^T k)               [B, 4096, 4096]
    out   = x + relu(fc_w @ (attn @ v^T)^T + fc_b)

Distribution: 8 cores = 4 batches x 2 query-block roles; role r owns
query blocks BLOCKS[r] (4 blocks of 512). Causal work is balanced by
giving role 0 global blocks [0,3,4,7] and role 1 blocks [1,2,5,6];
both roles run the identical SPMD program with per-slot key-tile
counts M_S = (8,16,24,32).

Key layout trick: each core's x columns are HOST-PERMUTED so that
pair p = [partner block, owned block]. Then the q projection for slot
p reads the owned half of pair p's SBUF tiles directly (no separate
xq DMA), and the causal mask structure becomes role-independent:
  - m-tiles of pairs < s: fully allowed (no mask)
  - m-tiles [M-8, M-4) (partner block): all-ones or all-zeros,
    applied as a per-core scalar flag multiply
  - m-tiles [M-4, M): the diagonal block; 4 SHARED triangular
    [128,512] masks (identical for every slot/role/core)

Softmax is computed without max-subtraction (scores are ~26+-5, far
from fp32 overflow): p = exp(s) * mask, normalized by a replicated
ones-matmul denominator computed per-oct (8 m-tiles) from a
VectorE-summed tile, so the PE does one [128,512] den matmul per 8.

Everything runs in bf16 on the PE (PSUM accumulates in f32). Score
absolute error from bf16 q/k is ~3e-3 (rounding errors add
incoherently across the 256 post-relu channels), i.e. ~0.3% on the
attention weights -- well inside the 2e-2 budget. The residual add
uses a separate f32 DMA of the owned columns.

The positional-embedding + bias rows enter via a third matmul per
projection with a [3, cols] pos tile (px, py, ones) -- a 3-partition
contraction costs the same PE time as any other but its DMA is tiny.
"""

import numpy as np

B = 4
C = 256
S = 64
N = S * S            # 4096
K = 256              # q/k/v channels
NBLK = 512           # query block width
NSLOT = 4            # owned query blocks per core
M_S = (8, 16, 24, 32)  # key-tile count per slot (128-wide key tiles)
BLOCKS = ((0, 3, 4, 7), (1, 2, 5, 6))  # role -> global block ids

_PROGRAM = None


def _build_program():
    import concourse.bacc as bacc
    import concourse.mybir as mybir
    import concourse.tile as tile

    F32 = mybir.dt.float32
    BF16 = mybir.dt.bfloat16
    Act = mybir.ActivationFunctionType

    nc = bacc.Bacc("TRN2", target_bir_lowering=False, debug=False)

    xb_d = nc.dram_tensor("xb", [C, N], BF16, kind="ExternalInput")
    posn_d = nc.dram_tensor("posn", [3, N], BF16, kind="ExternalInput")
    wk_d = nc.dram_tensor("wk", [C, K], BF16, kind="ExternalInput")
    wkp_d = nc.dram_tensor("wkp", [3, K], BF16, kind="ExternalInput")
    wv_d = nc.dram_tensor("wv", [C, K], BF16, kind="ExternalInput")
    wvp_d = nc.dram_tensor("wvp", [3, K], BF16, kind="ExternalInput")
    wq_d = nc.dram_tensor("wq", [C, K], BF16, kind="ExternalInput")
    wqp_d = nc.dram_tensor("wqp", [3, K], BF16, kind="ExternalInput")
    fcw_d = nc.dram_tensor("fcw", [C, C], BF16, kind="ExternalInput")
    fcb_d = nc.dram_tensor("fcb", [C, 1], F32, kind="ExternalInput")
    dmask_d = nc.dram_tensor("dmask", [4, 128, NBLK], BF16,
                             kind="ExternalInput")
    flags_d = nc.dram_tensor("flags", [128, NSLOT], F32,
                             kind="ExternalInput")
    ob_d = nc.dram_tensor("ones_b", [128, 128], BF16, kind="ExternalInput")
    xres_d = nc.dram_tensor("xres", [C, NSLOT * NBLK], F32,
                            kind="ExternalInput")
    out_d = nc.dram_tensor("out", [C, NSLOT * NBLK], F32,
                           kind="ExternalOutput")

    with tile.TileContext(nc) as tc:
        with (
            tc.tile_pool(name="wts", bufs=1) as wts,
            tc.tile_pool(name="xp", bufs=2) as xp,
            tc.tile_pool(name="kqv_p", bufs=1) as kqv_p,
            tc.tile_pool(name="xr_p", bufs=1) as xr_p,
            tc.tile_pool(name="ex_p", bufs=9) as ex_p,
            tc.tile_pool(name="ds_p", bufs=2) as ds_p,
            tc.tile_pool(name="o_p", bufs=4) as o_p,
            tc.tile_pool(name="rb_p", bufs=2) as rb_p,
            tc.tile_pool(name="tr_p", bufs=3) as tr_p,
            tc.tile_pool(name="ps_sc", bufs=4, space="PSUM") as ps_sc,
            tc.tile_pool(name="ps_out", bufs=1, space="PSUM") as ps_out,
            tc.tile_pool(name="ps_den", bufs=1, space="PSUM") as ps_den,
            tc.tile_pool(name="ps_fc", bufs=1, space="PSUM") as ps_fc,
        ):
            def wtile(dram, r0, rn, dt, tag):
                t = wts.tile([rn, dram.shape[1]], dt, tag=tag, name=tag)
                nc.sync.dma_start(t[:], dram[r0:r0 + rn, :])
                return t

            # weights needed by phase A first (k, v) so PE can start early
            wk_t = [wtile(wk_d, 0, 128, BF16, "wk0"),
                    wtile(wk_d, 128, 128, BF16, "wk1"),
                    wtile(wkp_d, 0, 3, BF16, "wk2")]
            wv_t = [wtile(wv_d, 0, 128, BF16, "wv0"),
                    wtile(wv_d, 128, 128, BF16, "wv1"),
                    wtile(wvp_d, 0, 3, BF16, "wv2")]

            k_sb = [[None] * 8 for _ in range(2)]
            vT_sb = [None] * 32
            q_sb = [[None] * NSLOT for _ in range(2)]
            wq_t = []  # filled after pair-0 DMAs

            # ---- phase A: per pair p, DMA x cols [1024p, 1024(p+1)),
            # then k and vT for both halves, then q for slot p from the
            # owned (second) half while the tiles are still in SBUF ----
            def emit_pair(p):
                cs = slice(2 * NBLK * p, 2 * NBLK * (p + 1))
                xt = []
                for ci in range(2):
                    t = xp.tile([128, 2 * NBLK], BF16, tag=f"xb{ci}",
                                name=f"xb{ci}_{p}")
                    nc.sync.dma_start(t[:], xb_d[128 * ci:128 * (ci + 1), cs])
                    xt.append(t)
                tp = xp.tile([3, 2 * NBLK], BF16, tag="pos", name=f"pos_{p}")
                nc.sync.dma_start(tp[:], posn_d[:, cs])
                xt.append(tp)

                for li, nb in enumerate((2 * p, 2 * p + 1)):
                    ls = slice(NBLK * li, NBLK * (li + 1))
                    for kt in range(2):
                        kts = slice(128 * kt, 128 * (kt + 1))
                        pk = ps_sc.tile([128, NBLK], F32, tag="sc",
                                        name=f"pk{kt}_{nb}")
                        for ci in range(3):
                            nc.tensor.matmul(pk[:], wk_t[ci][:, kts],
                                             xt[ci][:, ls],
                                             start=(ci == 0), stop=(ci == 2))
                        kt_sb = kqv_p.tile([128, NBLK], BF16,
                                           tag=f"k{kt}_{nb}",
                                           name=f"k{kt}_{nb}")
                        nc.scalar.activation(kt_sb[:], pk[:], Act.Relu)
                        k_sb[kt][nb] = kt_sb
                for li, nb in enumerate((2 * p, 2 * p + 1)):
                    for sub in range(4):
                        i = 4 * nb + sub
                        ss = slice(NBLK * li + 128 * sub,
                                   NBLK * li + 128 * (sub + 1))
                        pv = ps_sc.tile([128, K], F32, tag="sc",
                                        name=f"pv{i}")
                        for ci in range(3):
                            nc.tensor.matmul(pv[:], xt[ci][:, ss],
                                             wv_t[ci][:],
                                             start=(ci == 0), stop=(ci == 2))
                        vt_sb = kqv_p.tile([128, K], BF16, tag=f"v{i}",
                                           name=f"v{i}")
                        nc.scalar.activation(vt_sb[:], pv[:], Act.Relu)
                        vT_sb[i] = vt_sb
                # q for slot p from the owned (second) half
                os_ = slice(NBLK, 2 * NBLK)
                for kt in range(2):
                    kts = slice(128 * kt, 128 * (kt + 1))
                    pq = ps_sc.tile([128, NBLK], F32, tag="sc",
                                    name=f"pq{kt}_{p}")
                    for ci in range(3):
                        nc.tensor.matmul(pq[:], wq_t[ci][:, kts],
                                         xt[ci][:, os_],
                                         start=(ci == 0), stop=(ci == 2))
                    qt = kqv_p.tile([128, NBLK], BF16, tag=f"q{kt}_{p}",
                                    name=f"q{kt}_{p}")
                    nc.scalar.activation(qt[:], pq[:], Act.Relu)
                    q_sb[kt][p] = qt

            # pair-0 x DMAs first, then the smaller secondary inputs, so
            # the first k matmul's inputs land earliest
            cs0 = slice(0, 2 * NBLK)
            xt0 = []
            for ci in range(2):
                t = xp.tile([128, 2 * NBLK], BF16, tag=f"xb{ci}",
                            name=f"xb{ci}_0")
                nc.sync.dma_start(t[:], xb_d[128 * ci:128 * (ci + 1), cs0])
                xt0.append(t)
            tp0 = xp.tile([3, 2 * NBLK], BF16, tag="pos", name="pos_0")
            nc.sync.dma_start(tp0[:], posn_d[:, cs0])
            xt0.append(tp0)

            wq_t.extend([wtile(wq_d, 0, 128, BF16, "wq0"),
                         wtile(wq_d, 128, 128, BF16, "wq1"),
                         wtile(wqp_d, 0, 3, BF16, "wq2")])
            dmask_t = []
            for t_ in range(4):
                mt = wts.tile([128, NBLK], BF16, tag=f"dm{t_}",
                              name=f"dm{t_}")
                nc.sync.dma_start(mt[:], dmask_d[t_])
                dmask_t.append(mt)
            flags_t = wtile(flags_d, 0, 128, F32, "flags")
            ones_b = wtile(ob_d, 0, 128, BF16, "ones_b")
            fcw_t = [wtile(fcw_d, 0, 128, BF16, "fcw0"),
                     wtile(fcw_d, 128, 128, BF16, "fcw1")]
            fcb_t = [wtile(fcb_d, 0, 128, F32, "fcb0"),
                     wtile(fcb_d, 128, 128, F32, "fcb1")]

            # emit pair-0 compute reusing the already-started DMAs
            def emit_pair0_compute(xt):
                for li, nb in enumerate((0, 1)):
                    ls = slice(NBLK * li, NBLK * (li + 1))
                    for kt in range(2):
                        kts = slice(128 * kt, 128 * (kt + 1))
                        pk = ps_sc.tile([128, NBLK], F32, tag="sc",
                                        name=f"pk{kt}_{nb}")
                        for ci in range(3):
                            nc.tensor.matmul(pk[:], wk_t[ci][:, kts],
                                             xt[ci][:, ls],
                                             start=(ci == 0), stop=(ci == 2))
                        kt_sb = kqv_p.tile([128, NBLK], BF16,
                                           tag=f"k{kt}_{nb}",
                                           name=f"k{kt}_{nb}")
                        nc.scalar.activation(kt_sb[:], pk[:], Act.Relu)
                        k_sb[kt][nb] = kt_sb
                for li, nb in enumerate((0, 1)):
                    for sub in range(4):
                        i = 4 * nb + sub
                        ss = slice(NBLK * li + 128 * sub,
                                   NBLK * li + 128 * (sub + 1))
                        pv = ps_sc.tile([128, K], F32, tag="sc",
                                        name=f"pv{i}")
                        for ci in range(3):
                            nc.tensor.matmul(pv[:], xt[ci][:, ss],
                                             wv_t[ci][:],
                                             start=(ci == 0), stop=(ci == 2))
                        vt_sb = kqv_p.tile([128, K], BF16, tag=f"v{i}",
                                           name=f"v{i}")
                        nc.scalar.activation(vt_sb[:], pv[:], Act.Relu)
                        vT_sb[i] = vt_sb
                os_ = slice(NBLK, 2 * NBLK)
                for kt in range(2):
                    kts = slice(128 * kt, 128 * (kt + 1))
                    pq = ps_sc.tile([128, NBLK], F32, tag="sc",
                                    name=f"pq{kt}_0")
                    for ci in range(3):
                        nc.tensor.matmul(pq[:], wq_t[ci][:, kts],
                                         xt[ci][:, os_],
                                         start=(ci == 0), stop=(ci == 2))
                    qt = kqv_p.tile([128, NBLK], BF16, tag=f"q{kt}_0",
                                    name=f"q{kt}_0")
                    nc.scalar.activation(qt[:], pq[:], Act.Relu)
                    q_sb[kt][0] = qt

            emit_pair0_compute(xt0)
            for _p in range(1, 4):
                emit_pair(_p)

            # residual x (f32, owned columns), late: consumed by finalize
            xres_t = []
            for ot in range(2):
                t = xr_p.tile([128, NSLOT * NBLK], F32, tag=f"xr{ot}",
                              name=f"xr{ot}")
                nc.sync.dma_start(t[:], xres_d[128 * ot:128 * (ot + 1), :])
                xres_t.append(t)

            # ---- phase B: attention + fc per slot ----
            def finalize_slot(s, po, pd):
                """normalize slot s, fc, relu, residual, dma out."""
                rb_sb = rb_p.tile([128, NBLK], F32, tag="rb", name=f"rb{s}")
                nc.vector.reciprocal_approx_fast(rb_sb[:], pd[:])
                o_sb = []
                for vt in range(2):
                    ot_ = o_p.tile([128, NBLK], BF16, tag="o",
                                   name=f"o{vt}_{s}")
                    nc.vector.tensor_mul(ot_[:], po[vt][:], rb_sb[:])
                    o_sb.append(ot_)
                for ot in range(2):
                    pfc = ps_fc.tile([128, NBLK], F32, tag="fc",
                                     name=f"pfc{ot}_{s}")
                    for vt in range(2):
                        nc.tensor.matmul(
                            pfc[:], fcw_t[vt][:, 128 * ot:128 * (ot + 1)],
                            o_sb[vt][:], start=(vt == 0), stop=(vt == 1))
                    t_sb = tr_p.tile([128, NBLK], F32, tag=f"t{ot}",
                                     name=f"t{ot}_{s}")
                    nc.scalar.activation(t_sb[:], pfc[:], Act.Relu,
                                         bias=fcb_t[ot][:])
                    r_sb = tr_p.tile([128, NBLK], F32, tag=f"r{ot}",
                                     name=f"r{ot}_{s}")
                    nc.vector.tensor_add(
                        r_sb[:], t_sb[:],
                        xres_t[ot][:, NBLK * s:NBLK * (s + 1)])
                    nc.sync.dma_start(
                        out_d[128 * ot:128 * (ot + 1),
                              NBLK * s:NBLK * (s + 1)], r_sb[:])

            pending = None  # deferred finalize of previous slot
            for s in range(NSLOT):
                M = M_S[s]
                po = [ps_out.tile([128, NBLK], F32, tag=f"o{vt}",
                                  name=f"po{vt}_{s}") for vt in range(2)]
                pd = ps_den.tile([128, NBLK], F32, tag="den", name=f"pd{s}")
                ex_tiles = [None] * M
                dq_tiles = {}

                def emit_scores(i, s=s, ex_tiles=ex_tiles, M=M):
                    # scores^T tile [128 keys, 512 queries]
                    psc = ps_sc.tile([128, NBLK], F32, tag="sc",
                                     name=f"psc{s}_{i}")
                    for kt in range(2):
                        nc.tensor.matmul(
                            psc[:],
                            k_sb[kt][i // 4][:, 128 * (i % 4):128 * (i % 4 + 1)],
                            q_sb[kt][s][:], start=(kt == 0), stop=(kt == 1))
                    ex = ex_p.tile([128, NBLK], BF16, tag="ex",
                                   name=f"ex{s}_{i}")
                    nc.scalar.activation(ex[:], psc[:], Act.Exp)
                    if M - 8 <= i < M - 4:
                        # partner block: all-allowed or all-masked per core
                        nc.vector.tensor_scalar_mul(ex[:], ex[:],
                                                    flags_t[:, s:s + 1])
                    elif i >= M - 4:
                        # diagonal block: shared triangular mask
                        nc.vector.tensor_mul(ex[:], ex[:],
                                             dmask_t[i - (M - 4)][:])
                    ex_tiles[i] = ex

                def consume_quad(j, po=po, pd=pd, M=M, ex_tiles=ex_tiles,
                                 dq_tiles=dq_tiles, s=s):
                    for jj in range(j, j + 4):
                        e = ex_tiles[jj]
                        for vt in range(2):
                            nc.tensor.matmul(
                                po[vt][:],
                                vT_sb[jj][:, 128 * vt:128 * (vt + 1)],
                                e[:], start=(jj == 0), stop=(jj == M - 1))
                    # oct-summed denominator: one den matmul per 8 m-tiles
                    da = ds_p.tile([128, NBLK], BF16, tag="da",
                                   name=f"da{s}_{j}")
                    nc.vector.tensor_add(da[:], ex_tiles[j][:],
                                         ex_tiles[j + 1][:])
                    db = ds_p.tile([128, NBLK], BF16, tag="db",
                                   name=f"db{s}_{j}")
                    nc.vector.tensor_add(db[:], ex_tiles[j + 2][:],
                                         ex_tiles[j + 3][:])
                    dq = ds_p.tile([128, NBLK], BF16, tag="dq",
                                   name=f"dq{s}_{j}")
                    nc.vector.tensor_add(dq[:], da[:], db[:])
                    dq_tiles[j] = dq
                    if j % 8 == 4:
                        doct = ds_p.tile([128, NBLK], BF16, tag="do",
                                         name=f"do{s}_{j}")
                        nc.vector.tensor_add(doct[:], dq_tiles[j - 4][:],
                                             dq[:])
                        nc.tensor.matmul(pd[:], ones_b[:], doct[:],
                                         start=(j == 4), stop=(j == M - 4))
                    for jj in range(j, j + 4):
                        ex_tiles[jj] = None

                for ib in range(0, M, 4):
                    for i in range(ib, ib + 4):
                        emit_scores(i)
                    if ib == 4 and pending is not None:
                        finalize_slot(*pending)
                        pending = None
                    if ib >= 4:
                        consume_quad(ib - 4)
                consume_quad(M - 4)
                pending = (s, po, pd)

            finalize_slot(*pending)

    nc.compile()
    return nc


def _host_prep(x, q_w, q_b, k_w, k_b, v_w, v_b, fc_w, fc_b):
    """Build the per-core input maps."""
    import ml_dtypes
    f32 = np.float32
    bf16 = ml_dtypes.bfloat16
    n = np.arange(N)
    px = ((n // S) / S).astype(f32)
    py = ((n % S) / S).astype(f32)
    pos3 = np.stack([px, py, np.ones(N, f32)])   # [3, N] (incl bias channel)

    def wpair(w, b):
        wt = w.astype(f32).T            # [258, 256]
        main = np.ascontiguousarray(wt[:C]).astype(bf16)
        posb = np.ascontiguousarray(
            np.concatenate([wt[C:], b.astype(f32)[None, :]], 0)).astype(bf16)
        return main, posb

    wk, wkp = wpair(k_w, k_b)
    wv, wvp = wpair(v_w, v_b)
    wq, wqp = wpair(q_w, q_b)

    # shared diagonal triangular masks [4, 128, 512]
    mm = np.arange(128)[:, None]
    nn = np.arange(NBLK)[None, :]
    dmask = np.stack([(128 * t + mm <= nn) for t in range(4)]).astype(bf16)

    shared = {
        "wk": wk, "wkp": wkp, "wv": wv, "wvp": wvp, "wq": wq, "wqp": wqp,
        "fcw": np.ascontiguousarray(fc_w.astype(f32).T).astype(bf16),
        "fcb": np.ascontiguousarray(fc_b.astype(f32).reshape(C, 1)),
        "dmask": dmask,
        "ones_b": np.ones((128, 128), bf16),
    }

    in_maps = []
    for c in range(8):
        b, r = c // 2, c % 2
        xb = x[b].reshape(C, N).astype(f32)
        # permuted block order: pair p = [partner, owned]
        blocks_perm = []
        flags = np.zeros(NSLOT, f32)
        for p in range(NSLOT):
            j = BLOCKS[r][p]
            o = 2 * p + (1 - (j - 2 * p))   # partner block of the pair
            blocks_perm += [o, j]
            flags[p] = 1.0 if j == 2 * p + 1 else 0.0
        cols = np.concatenate(
            [np.arange(NBLK * blk, NBLK * (blk + 1)) for blk in blocks_perm])
        own_cols = np.concatenate(
            [np.arange(NBLK * j, NBLK * (j + 1)) for j in BLOCKS[r]])
        in_maps.append(dict(
            shared,
            xb=np.ascontiguousarray(xb[:, cols]).astype(bf16),
            posn=np.ascontiguousarray(pos3[:, cols]).astype(bf16),
            flags=np.ascontiguousarray(np.broadcast_to(flags[None, :], (128, NSLOT))),
            xres=np.ascontiguousarray(xb[:, own_cols]),
        ))
    return in_maps


def _gather(results):
    out = np.empty((B, C, N), np.float32)
    for c in range(8):
        b, r = c // 2, c % 2
        oc = results[c]["out"]
        for s, j in enumerate(BLOCKS[r]):
            out[b][:, NBLK * j:NBLK * (j + 1)] = oc[:, NBLK * s:NBLK * (s + 1)]
    return out.reshape(B, C, S, S)


def run(trace=False, **inputs):
    from concourse import bass_utils
    global _PROGRAM
    if _PROGRAM is None:
        _PROGRAM = _build_program()
    in_maps = _host_prep(**inputs)
    res = bass_utils.run_bass_kernel_spmd(
        _PROGRAM, in_maps, list(range(8)), trace=trace)
    return _gather(res.results), res


def kernel(**inputs):
    out, _ = run(trace=False, **inputs)
    return out
